# revision 25
# baseline (speedup 1.0000x reference)
"""Trainium2 Bass kernel for RoPE causal multi-head attention (one nn.Module).

Reference (fp32): q,k,v = x @ {Wq,Wk,Wv}.T (16 heads of 64); rope(q,k);
out = softmax(q k^T / 8 + mask) @ v merged @ Wo.T.  B=2, L=2048, D=1024.

Sharding over 8 NeuronCores: data-parallel on batch (B=2) x tensor-parallel on
heads (4 groups of 4). Each core computes its 4 heads plus a partial output
projection against its Wo column slice; the host sums 4 partials per batch
element (0.02% of FLOPs) and re-assembles the full output.

Device dataflow per core (all matmuls float32r = full PE rate, ~1.6e-4 rel):
  - host pre-transposes AND pre-tiles every input into the exact SBUF layout
    so each DMA is large contiguous per-partition runs and the device never
    transposes
  - q,k are produced transposed ([head_dim, L]); v in natural [L, head_dim]
  - rope fused into the projection: q' = (P@wq x)*cos + R@((P@wq x)*sin) with
    R a constant pair-rotation matrix applied on the PE
  - scores are computed transposed ([l_k, l_q]) so the softmax denominator
    and attn@v both contract over l_k on the PE; the denominator falls out of
    an extra ones-column appended to v (row 64 of the attn@v accumulator)
  - the head pair shares PE row-groups 0-63/64-127 via tile_position; their
    two score PSUM banks form one [128, 2, 512] tile so a single ACT exp
    (scale=0.125 folded in) covers both heads
  - causal masking with NO GpSimd (GpSimd ops serialized ~3.4us each and
    caused HAM oscillation): above-diagonal l_k tiles are skipped, boundary
    tiles compute only the 256-aligned live column range (keeps fp32r matmul
    free-dim >= 256 = full rate) and get one DVE multiply with a host
    [tri|ones] / [zeros|tri] strip; a zero mask skips masking; any other mask
    falls back to multiplying exp(scores) by a host-provided exp(mask^T)
  - softmax normalization: DVE reciprocal of the denominator row, broadcast
    to 64 partitions via a K=1 PE matmul (ones outer product), DVE multiply
  - the attention inner loop is software-pipelined one l_k tile deep
    (scores of tile k+1 issue before attn@v of tile k) so the in-order PE
    stream never blocks on the ACT exp latency

build_kernel(mask_mode, repeat): `repeat` wraps the body in an on-device
tc.For_i loop, used only for hardware timing (no NTFF tracing on axon here);
per-iteration = (warm_wall[R] - warm_wall[1]) / (R - 1).
"""

import numpy as np

import concourse.bass as bass
import concourse.mybir as mybir
from concourse import bacc
import concourse.tile as tile
from concourse.bass_utils import run_bass_kernel_spmd

F32 = mybir.dt.float32
F32R = mybir.dt.float32r
EXP = mybir.ActivationFunctionType.Exp

B, L, D, NH, HD = 2, 2048, 1024, 16, 64
HPC = NH // 4          # heads per core = 4
DQ = HPC * HD          # per-core projected width = 256
NB = L // 512          # 512-wide l_q blocks = 4
NT = L // 128          # 128-wide l_k tiles = 16


def build_kernel(mask_mode: str, repeat: int = 1, norm: str = "exact") -> bass.Bass:
    nc = bacc.Bacc(None)
    xT = nc.declare_dram_parameter("xT", [NB, 128, 8, 512], F32R, isOutput=False)
    wqT = nc.declare_dram_parameter("wqT", [128, 8, DQ], F32R, isOutput=False)
    wkT = nc.declare_dram_parameter("wkT", [128, 8, DQ], F32R, isOutput=False)
    wvT = nc.declare_dram_parameter("wvT", [128, 8, DQ], F32R, isOutput=False)
    woT = nc.declare_dram_parameter("woT", [128, 2, D], F32R, isOutput=False)
    cosT = nc.declare_dram_parameter("cosT", [128, L], F32, isOutput=False)
    sinT = nc.declare_dram_parameter("sinT", [128, L], F32, isOutput=False)
    rT = nc.declare_dram_parameter("rT", [128, 128], F32R, isOutput=False)
    masks = nc.declare_dram_parameter("masks", [128, 2, 2, 256], F32, isOutput=False)
    ones = nc.declare_dram_parameter("ones", [128, 64], F32R, isOutput=False)
    if mask_mode == "general":
        emT = nc.declare_dram_parameter("emT", [L, L], F32, isOutput=False)
    outT = nc.declare_dram_parameter("outT", [D, L], F32, isOutput=True)

    with tile.TileContext(nc) as tc:
        with (
            tc.tile_pool(name="const", bufs=1) as const,
            tc.tile_pool(name="persist", bufs=1) as persist,
            tc.tile_pool(name="psc", bufs=2, space="PSUM") as psc,
            tc.tile_pool(name="pav", bufs=2, space="PSUM") as pav,
            tc.tile_pool(name="psb", bufs=2, space="PSUM") as psb,
            tc.tile_pool(name="xs", bufs=2) as xs,
            tc.tile_pool(name="wp", bufs=1) as wp,
            tc.tile_pool(name="qs", bufs=2) as qs_pool,
            tc.tile_pool(name="ep", bufs=4 if mask_mode != "general" else 3) as ep,
            tc.tile_pool(name="em", bufs=2) as emp,
            tc.tile_pool(name="rp", bufs=2) as rp,
            tc.tile_pool(name="oc", bufs=2) as ocp,
        ):
            cos_sb = const.tile([128, L], F32)
            sin_sb = const.tile([128, L], F32)
            rT_sb = const.tile([128, 128], F32R)
            masks_sb = const.tile([128, 2, 2, 256], F32)
            wo_sb = const.tile([128, 2, D], F32R)
            ones_sb = const.tile([128, 64], F32R)

            q_sb = persist.tile([128, 2, L], F32R)
            k_sb = persist.tile([128, 2, L], F32R)
            v_sb = persist.tile([128, NT, HPC, HD + 1], F32R)
            o_sb = persist.tile([128, 2, L], F32R)

            wq_sb = wp.tile([128, 8, DQ], F32R)
            wk_sb = wp.tile([128, 8, DQ], F32R)
            wv_sb = wp.tile([128, 8, DQ], F32R)

            def _emit_body():
                state = {}
                xt0 = xs.tile([128, 8, 512], F32R, tag="xt")
                for kc in range(8):
                    nc.sync.dma_start(wq_sb[:, kc], wqT[:, kc])
                    nc.sync.dma_start(xt0[:, kc], xT[0:1, :, kc, :].rearrange("a p f -> p (a f)"))
                nc.sync.dma_start(wk_sb[:], wkT[:])
                nc.sync.dma_start(wv_sb[:], wvT[:])
                nc.sync.dma_start(cos_sb[:, 0:512], cosT[:, 0:512])
                nc.sync.dma_start(sin_sb[:, 0:512], sinT[:, 0:512])
                nc.sync.dma_start(rT_sb[:], rT[:])
                nc.sync.dma_start(masks_sb[:], masks[:])
                nc.sync.dma_start(ones_sb[:], ones[:])
                nc.vector.tensor_copy(
                    v_sb[:, :, :, HD : HD + 1],
                    ones_sb.rearrange("p (a b c) -> p a b c", a=NT, b=HPC, c=1),
                )
                for c in range(1, 4):
                    nc.sync.dma_start(cos_sb[:, c * 512 : (c + 1) * 512],
                                      cosT[:, c * 512 : (c + 1) * 512])
                    nc.sync.dma_start(sin_sb[:, c * 512 : (c + 1) * 512],
                                      sinT[:, c * 512 : (c + 1) * 512])
                nc.sync.dma_start(wo_sb[:], woT[:])
                state[("xt", 0)] = xt0

                def qkv_units(n):
                    sl = slice(n * 512, (n + 1) * 512)
                    xt = state[("xt", n)]
                    us = []

                    def get_qs():
                        key = ("qs", n)
                        if key not in state:
                            state[key] = qs_pool.tile([128, 4, 512], F32R,
                                                      tag="qs", name=f"qs{n}")
                        return state[key]

                    for i, (w_sb, dst) in enumerate(((wq_sb, q_sb), (wk_sb, k_sb))):
                        for t in range(2):
                            def u_qk(i=i, t=t, w_sb=w_sb, dst=dst):
                                qs_blk = get_qs()
                                p = psb.tile([128, 512], F32, tag="ps")
                                for kc in range(8):
                                    nc.tensor.matmul(
                                        p[:], w_sb[:, kc, t * 128 : (t + 1) * 128],
                                        xt[:, kc, :], start=(kc == 0), stop=(kc == 7))
                                nc.vector.tensor_mul(out=dst[:, t, sl], in0=p[:],
                                                     in1=cos_sb[:, sl])
                                nc.vector.tensor_mul(out=qs_blk[:, 2 * i + t],
                                                     in0=p[:], in1=sin_sb[:, sl])
                            us.append(u_qk)
                    for lt in range(4 * n, 4 * n + 4):
                        def u_v(lt=lt):
                            p = psb.tile([128, 512], F32, tag="ps")
                            for kc in range(8):
                                nc.tensor.matmul(
                                    p[:, :DQ],
                                    xt[:, kc, (lt - 4 * n) * 128 : (lt - 4 * n + 1) * 128],
                                    wv_sb[:, kc, :], start=(kc == 0), stop=(kc == 7))
                            nc.vector.tensor_copy(
                                v_sb[:, lt, :, 0:HD],
                                p[:, :DQ].rearrange("p (h e) -> p h e", h=HPC))
                        us.append(u_v)
                    for i, dst in enumerate((q_sb, k_sb)):
                        for t in range(2):
                            def u_rope(i=i, t=t, dst=dst):
                                qs_blk = get_qs()
                                p = psb.tile([128, 512], F32, tag="ps")
                                nc.tensor.matmul(p[:], rT_sb[:], qs_blk[:, 2 * i + t],
                                                 start=True, stop=True)
                                nc.vector.tensor_add(out=dst[:, t, sl],
                                                     in0=dst[:, t, sl], in1=p[:])
                            us.append(u_rope)
                    return us

                def attn_steps(n):
                    sl = slice(n * 512, (n + 1) * 512)

                    def make_pair(hp):
                        heads = (2 * hp, 2 * hp + 1)
                        t = hp
                        if mask_mode == "causal":
                            lk_tiles = list(range(0, 4 * n + 4))
                        else:
                            lk_tiles = list(range(NT))
                        last = lk_tiles[-1]
                        ctx = {"prev": None, "avps": None}

                        def get_avps():
                            if ctx["avps"] is None:
                                ctx["avps"] = {
                                    h: pav.tile([128, 512], F32, tag="avp",
                                                name=f"avp_{n}_{h}")
                                    for h in heads
                                }
                            return ctx["avps"]

                        def emit_scores(lk):
                            j = lk - 4 * n
                            boundary = mask_mode == "causal" and j >= 0
                            c0e = 256 if (boundary and j >= 2) else 0
                            sc = psc.tile([128, 2, 512], F32, tag="sc",
                                          name=f"sc_{n}_{hp}_{lk}")
                            for hi, h in enumerate(heads):
                                ro = 64 * hi
                                nc.tensor.matmul(
                                    sc[:, hi, c0e:],
                                    k_sb[ro : ro + 64, t, lk * 128 : (lk + 1) * 128],
                                    q_sb[ro : ro + 64, t, n * 512 + c0e : (n + 1) * 512],
                                    start=True, stop=True, tile_position=(ro, 0))
                            e = ep.tile([128, 2, 512], F32R, tag="e",
                                        name=f"e_{n}_{hp}_{lk}")
                            nc.scalar.activation(e[:, :, c0e:], sc[:, :, c0e:],
                                                 EXP, scale=0.125)
                            if boundary:
                                nc.vector.tensor_mul(
                                    out=e[:, :, c0e : c0e + 256],
                                    in0=e[:, :, c0e : c0e + 256],
                                    in1=masks_sb[:, j % 2])
                            elif mask_mode == "general":
                                em = emp.tile([128, 512], F32, tag="em")
                                nc.sync.dma_start(
                                    em[:],
                                    emT[lk * 128 : (lk + 1) * 128, n * 512 : (n + 1) * 512])
                                for hi in range(2):
                                    nc.vector.tensor_mul(out=e[:, hi], in0=e[:, hi],
                                                         in1=em[:])
                            return e, c0e, lk

                        def emit_av(st):
                            e, c0e, lk = st
                            avps = get_avps()
                            for hi, h in enumerate(heads):
                                nc.tensor.matmul(
                                    avps[h][0 : HD + 1, c0e:],
                                    v_sb[:, lk, h, :],
                                    e[:, hi, c0e:],
                                    start=(lk == lk_tiles[0]), stop=(lk == last))

                        def step(lk):
                            def s():
                                cur = emit_scores(lk)
                                if ctx["prev"] is not None:
                                    emit_av(ctx["prev"])
                                ctx["prev"] = cur
                            return s

                        def tail():
                            """av(last) + one ACT evacuation per head (frees the
                            avp PSUM banks); the rest of the norm chain is
                            deferred into the zip unit stream for latency
                            hiding."""
                            emit_av(ctx["prev"])
                            avps = get_avps()
                            for hi, h in enumerate(heads):
                                ro = 64 * hi
                                avo = rp.tile([128, 512], F32, tag="avo", bufs=5)
                                nc.scalar.copy(avo[0 : HD + 1, :],
                                               avps[h][0 : HD + 1, :])

                                def u_recip(avo=avo, h=h):
                                    dnm = rp.tile([1, 512], F32, tag="dnm")
                                    nc.scalar.copy(dnm[:], avo[HD : HD + 1, :])
                                    rec0 = rp.tile([1, 512], F32, tag="rec0")
                                    nc.vector.reciprocal_approx_fast(rec0[:], dnm[:])
                                    rec = rp.tile([1, 512], F32R, tag="rec")
                                    with nc.allow_low_precision(reason="f32r rounding"):
                                        nc.vector.tensor_copy(rec[:], rec0[:])
                                    state[("rec", n, h)] = rec

                                def u_bps(h=h):
                                    rec = state.pop(("rec", n, h))
                                    bps = psb.tile([128, 512], F32, tag="ps",
                                                   name=f"bps_{n}_{h}")
                                    nc.tensor.matmul(bps[0:64, :], ones_sb[0:1, :],
                                                     rec[:], start=True, stop=True)
                                    state[("bps", n, h)] = bps

                                def u_omul(avo=avo, ro=ro, h=h):
                                    bps = state.pop(("bps", n, h))
                                    nc.vector.tensor_mul(
                                        out=o_sb[ro : ro + 64, t, sl],
                                        in0=avo[0:HD, :],
                                        in1=bps[0:64, :])

                                chain_q.extend([u_recip, u_bps, u_omul])
                        return [step(lk) for lk in lk_tiles] + [tail]

                    return make_pair(0) + make_pair(1)

                def oproj_units(n):
                    sl = slice(n * 512, (n + 1) * 512)
                    us = []
                    for m in range(8):
                        def u(m=m):
                            p = psb.tile([128, 512], F32, tag="ps")
                            for kc in range(2):
                                nc.tensor.matmul(
                                    p[:], wo_sb[:, kc, m * 128 : (m + 1) * 128],
                                    o_sb[:, kc, sl], start=(kc == 0), stop=(kc == 1))
                            oc = ocp.tile([128, 512], F32, tag="oc")
                            nc.vector.tensor_copy(oc[:], p[:])
                            nc.sync.dma_start(outT[m * 128 : (m + 1) * 128, sl], oc[:])
                        us.append(u)
                    return us

                def zip_emit(steps, units):
                    ns = max(1, len(steps))
                    nu = len(units)
                    ui = 0
                    for si, s in enumerate(steps):
                        s()
                        target = (si + 1) * nu // ns
                        while ui < target:
                            units[ui]()
                            ui += 1
                    while ui < nu:
                        units[ui]()
                        ui += 1

                chain_q = []

                def drain_chain():
                    us = list(chain_q)
                    chain_q.clear()
                    return us

                for u in qkv_units(0):
                    u()
                for n in range(1, NB):
                    xt = xs.tile([128, 8, 512], F32R, tag="xt")
                    nc.sync.dma_start(xt[:], xT[n : n + 1].rearrange("a p o f -> p o (a f)"))
                    state[("xt", n)] = xt
                    units = drain_chain() + qkv_units(n)
                    if n >= 2:
                        units = units + oproj_units(n - 2)
                    zip_emit(attn_steps(n - 1), units)
                zip_emit(attn_steps(NB - 1), drain_chain() + oproj_units(NB - 2))
                for u in drain_chain():
                    u()
                for u in oproj_units(NB - 1):
                    u()

            if repeat == 1:
                _emit_body()
            else:
                with tc.For_i(0, repeat, 1):
                    _emit_body()
    nc.finalize()
    return nc

_compiled = {}


def _get_kernel(mask_mode):
    if mask_mode not in _compiled:
        _compiled[mask_mode] = build_kernel(mask_mode)
    return _compiled[mask_mode]


def kernel(x, freqs, attention_mask, Wq, Wk, Wv, Wo, _trace=False, _trace_kwargs=None):
    x = np.asarray(x, dtype=np.float32)
    freqs = np.asarray(freqs, dtype=np.float32)
    mask = np.asarray(attention_mask, dtype=np.float32).reshape(L, L)
    Wq, Wk, Wv, Wo = (np.asarray(w, dtype=np.float32) for w in (Wq, Wk, Wv, Wo))

    # mask classification
    causal_ref = np.where(np.tri(L, dtype=bool), 0.0, -1e9).astype(np.float32)
    if not mask.any():
        mask_mode = "zero"
    elif np.array_equal(mask, causal_ref):
        mask_mode = "causal"
    else:
        mask_mode = "general"

    # host-side shared prep
    fr, fi = freqs[..., 0], freqs[..., 1]            # [L, HD//2]
    cosE = np.repeat(fr, 2, axis=1).T                # [HD, L]
    sinE = np.repeat(fi, 2, axis=1).T
    cos128 = np.ascontiguousarray(np.concatenate([cosE, cosE], axis=0))  # [128, L]
    sin128 = np.ascontiguousarray(np.concatenate([sinE, sinE], axis=0))
    # rotation matrix: rot[2i] = -x[2i+1], rot[2i+1] = x[2i]; rT = R^T
    R = np.zeros((128, 128), dtype=np.float32)
    for i in range(64):
        R[2 * i, 2 * i + 1] = -1.0
        R[2 * i + 1, 2 * i] = 1.0
    rT = np.ascontiguousarray(R.T)
    # boundary-mask strips over the 256-wide live range of a causal boundary
    # tile: even j -> [tri|ones] (keep cc >= r), odd j -> [zeros|tri]
    # (keep cc >= r + 128); duplicated for the 2 heads of a PE pair.
    r_idx = np.arange(128)[:, None]
    cc = np.arange(256)[None, :]
    mTO = (cc >= r_idx).astype(np.float32)
    mZT = (cc >= r_idx + 128).astype(np.float32)
    masksA = np.ascontiguousarray(
        np.broadcast_to(
            np.stack([mTO, mZT], axis=1)[:, :, None, :], (128, 2, 2, 256)
        ).copy()
    )

    in_maps = []
    for c in range(8):
        b, g = divmod(c, 4)
        rows = slice(DQ * g, DQ * (g + 1))
        def tile_w(wt):  # [D, DQ] -> [128, 8, DQ]
            return np.ascontiguousarray(wt.reshape(8, 128, -1).transpose(1, 0, 2))
        xt_full = x[b].T  # [D, L]
        xt4 = np.ascontiguousarray(
            xt_full.reshape(8, 128, NB, 512).transpose(2, 1, 0, 3)
        )  # [NB, 128, 8, 512]
        wot = Wo[:, rows].T  # [DQ, D]
        m = {
            "xT": xt4,
            "wqT": tile_w(Wq[rows].T),
            "wkT": tile_w(Wk[rows].T),
            "wvT": tile_w(Wv[rows].T),
            "woT": np.ascontiguousarray(wot.reshape(2, 128, D).transpose(1, 0, 2)),
            "cosT": cos128,
            "sinT": sin128,
            "rT": rT,
            "masks": masksA,
            "ones": np.ones((128, 64), dtype=np.float32),
        }
        if mask_mode == "general":
            m["emT"] = np.ascontiguousarray(np.exp(mask).T)
        in_maps.append(m)

    nc = _get_kernel(mask_mode)
    kw = {}
    if _trace:
        kw = dict(trace=True, trace_kwargs=_trace_kwargs or {})
    res = run_bass_kernel_spmd(nc, in_maps, list(range(8)), **kw)
    out = np.empty((B, L, D), dtype=np.float32)
    for b in range(B):
        acc = res.results[4 * b]["outT"].astype(np.float32)
        for g in range(1, 4):
            acc = acc + res.results[4 * b + g]["outT"]
        out[b] = acc.T
    kernel.last_result = res
    return out
